# revision 30
# baseline (speedup 1.0000x reference)
# Trainium2 Bass kernel for nn_CFTAuxHead (bilinear 4x resize + bbox
# rasterization + MSE loss), data-parallel over batch across 8 NeuronCores.
#
# Math summary (per sample):
#   feat_up = A^T @ feat @ A  (A = exact 160->640 bilinear weight matrix)
#   heatmap = last-writer-wins paint of 128 axis-aligned rects (value z_n)
#   loss    = mean((feat_up - heatmap)^2) over all pixels
#
# Rasterization: per output row-tile, two paint matmuls over box-indicator
# rank-1 products with exponent-coded weights:
#   T_S = eps + sum_n 2^(n-64) [covered]   (bf16-exact powers of two)
#   T_A = sum_n z_n 2^(n-64) [covered]     (z in bf16)
# Decode (exact for coverage depth<=1; depth-2 error ~z*2^(j-k); measured
# total loss error ~1.6e-4 vs 2e-2 tolerance):
#   E      = T_S.bits & 0xFF800000             [DVE int, isolates 2^top]
#   Z.bits = (T_A.bits + 0x3F800000) - E       [DVE int sub = divide by 2^e]
# Loss: PE subtracts Z from the resized-feature PSUM tile via a negated
# fp16 identity matmul (F -= Z in-place), then ACT squares straight from
# PSUM with accum_out. Pool converts Z->fp16 for the PE subtract.
#
# Engine budget per [128,640] tile: DVE 2 passes (E, Z), Pool 1 (fp16 cvt),
# ACT 1 (square+accum), PE ~3800 cycles of paints. The resize runs as fp32r
# (step 1, >=256-wide splits) and bf16 (step 2) matmuls. U/V indicators are
# built on DVE in fp16 4x mode; the next sample's indicator prep is emitted
# between decode ops to fill DVE write-ack latency gaps. PSUM: four
# 2-bank tiles (S, A, F1, F2) -- F double-buffered by tag alternation.
#
import numpy as np

B, C_IN, H, W = 32, 1, 160, 160
UP = 4
HO, WO = H * UP, W * UP
NBOX = 128
NCORES = 8
SPC = B // NCORES  # samples per core
NPIX = float(B * HO * WO)

MASK_EXP = -8388608  # 0xFF800000 as signed int32
XBIAS = 0x3F800000
EPS = float(2.0 ** -65)

_CACHE = {}


def _resize_matrix():
    """Exact bilinear (half-pixel centers, edge-clamped) 160->640 matrix,
    matching jax.image.resize(method='bilinear') for upsampling."""
    n_in, n_out = H, HO
    scale = n_out / n_in
    x = (np.arange(n_out, dtype=np.float64) + 0.5) / scale - 0.5
    k = np.arange(n_in, dtype=np.float64)
    w = np.maximum(0.0, 1.0 - np.abs(x[None, :] - k[:, None]))  # [in, out]
    w = w / w.sum(axis=0, keepdims=True)
    return w.astype(np.float32)


def _build(krep=1):
    import concourse.bacc as bacc
    import concourse.mybir as mybir
    from concourse.tile import TileContext

    fp32 = mybir.dt.float32
    fp32r = mybir.dt.float32r
    bf16 = mybir.dt.bfloat16
    f16 = mybir.dt.float16
    i32 = mybir.dt.int32
    Alu = mybir.AluOpType
    AF = mybir.ActivationFunctionType

    nc = bacc.Bacc("TRN2", target_bir_lowering=False, debug=False,
                   enable_asserts=False, num_devices=NCORES)
    feat_d = nc.dram_tensor("feat", [SPC, H, W], fp32r, kind="ExternalInput")
    box_d = nc.dram_tensor("boxes", [SPC, NBOX, 5], fp32, kind="ExternalInput")
    amat_d = nc.dram_tensor("amat", [H, HO], fp32r, kind="ExternalInput")
    amatb_d = nc.dram_tensor("amatb", [H, HO], bf16, kind="ExternalInput")
    out_d = nc.dram_tensor("out", [1, 1], fp32, kind="ExternalOutput")

    with TileContext(nc, num_cores=NCORES) as tc:
        with tc.tile_pool(name="const", bufs=1) as cpool, \
             tc.tile_pool(name="samp", bufs=4) as spool, \
             tc.tile_pool(name="dec", bufs=4) as dpool, \
             tc.tile_pool(name="psS", bufs=1, space="PSUM") as poolS, \
             tc.tile_pool(name="psA", bufs=1, space="PSUM") as poolA, \
             tc.tile_pool(name="psF", bufs=1, space="PSUM") as poolF:

            # ---- box DMA first: it gates the DVE startup chain ----
            bxall = cpool.tile([128, 5 * SPC], fp32, tag="bxall")
            bsrc = box_d.ap().transpose([1, 2, 0])  # [NBOX, 5, SPC]
            nc.scalar.dma_start(bxall[:], bsrc)

            # ---- constants ----
            A0 = cpool.tile([128, HO], fp32r, tag="A0")
            A1 = cpool.tile([32, HO], fp32r, tag="A1")
            nc.sync.dma_start(A0[:], amat_d.ap()[0:128, :])
            nc.sync.dma_start(A1[:], amat_d.ap()[128:160, :])
            B0 = cpool.tile([128, HO], bf16, tag="B0")
            B1 = cpool.tile([32, HO], bf16, tag="B1")
            nc.scalar.dma_start(B0[:], amatb_d.ap()[0:128, :])
            nc.scalar.dma_start(B1[:], amatb_d.ap()[128:160, :])

            iota_i = cpool.tile([128, HO], i32, tag="ioti")
            nc.gpsimd.iota(iota_i[:], pattern=[[1, HO]], base=0,
                           channel_multiplier=0)
            iota_h = cpool.tile([128, HO], f16, tag="ioth")
            nc.gpsimd.tensor_copy(iota_h[:], iota_i[:])

            nidx_i = cpool.tile([128, 1], i32, tag="nidxi")
            nc.gpsimd.iota(nidx_i[:], pattern=[[1, 1]], base=0,
                           channel_multiplier=1)  # n = 0..127
            # wS_base = 2^(n-64) : bits = (n + 63) << 23
            wS_base = cpool.tile([128, 1], fp32, tag="wSb")
            nc.vector.tensor_scalar(wS_base[:].bitcast(i32), nidx_i[:], 63,
                                    None, Alu.add)
            nc.vector.tensor_scalar(wS_base[:].bitcast(i32),
                                    wS_base[:].bitcast(i32), 23, None,
                                    Alu.logical_shift_left)

            eps_row = cpool.tile([1, NBOX], bf16, tag="epsr")
            nc.gpsimd.memset(eps_row[:], EPS)
            ones_row = cpool.tile([1, HO], bf16, tag="onesr")
            nc.gpsimd.memset(ones_row[:], 1.0)
            ones_col = cpool.tile([128, 1], fp32, tag="onesc")
            nc.gpsimd.memset(ones_col[:], 1.0)

            # negated identity (fp32r) for the PE Z-subtract
            icol_i = cpool.tile([128, 128], i32, tag="icoli")
            nc.gpsimd.iota(icol_i[:], pattern=[[1, 128]], base=0,
                           channel_multiplier=0)
            icol_f = cpool.tile([128, 128], fp32, tag="icolf")
            nc.vector.tensor_copy(icol_f[:], icol_i[:])
            nidx_f = cpool.tile([128, 1], fp32, tag="nidxf")
            nc.vector.tensor_copy(nidx_f[:], nidx_i[:])
            negI = cpool.tile([128, 128], f16, tag="negI")
            nc.vector.tensor_scalar(negI[:], icol_f[:], nidx_f[:], None,
                                    Alu.is_equal)
            nc.vector.tensor_scalar(negI[:], negI[:], -1.0, None, Alu.mult)

            accbuf = cpool.tile([128, krep * SPC * 5], fp32, tag="acc")

            # all samples' features in two DMAs: [h, s*W + w]
            F0all = cpool.tile([128, SPC * W], fp32r, tag="F0all")
            F1all = cpool.tile([32, SPC * W], fp32r, tag="F1all")
            fsrc = feat_d.ap().transpose([1, 0, 2])  # [H, SPC, W]
            nc.sync.dma_start(F0all[:], fsrc[0:128])
            nc.sync.dma_start(F1all[:], fsrc[128:160])

            # ---- batched box prep on Pool/DVE: [128, SPC] per field ----
            xq = bxall[:, 0 * SPC:1 * SPC]
            yq = bxall[:, 1 * SPC:2 * SPC]
            zq = bxall[:, 2 * SPC:3 * SPC]
            wq = bxall[:, 3 * SPC:4 * SPC]
            lq = bxall[:, 4 * SPC:5 * SPC]

            def floors(specs):
                """Interleaved floor(x) chains: [(src_ap, tag, scale), ...]"""
                sps, tis, tfs, ms, fls = [], [], [], [], []
                for src_ap, tagp, scale in specs:
                    if scale is not None:
                        sc = cpool.tile([128, SPC], fp32, tag=tagp + "_s")
                        nc.vector.tensor_scalar(sc[:], src_ap, scale, None,
                                                Alu.mult)
                        sps.append(sc[:])
                    else:
                        sps.append(src_ap)
                    tis.append(cpool.tile([128, SPC], i32, tag=tagp + "_i", name=tagp + "_i"))
                    tfs.append(cpool.tile([128, SPC], fp32, tag=tagp + "_f", name=tagp + "_f"))
                    ms.append(cpool.tile([128, SPC], fp32, tag=tagp + "_m", name=tagp + "_m"))
                    fls.append(cpool.tile([128, SPC], fp32, tag=tagp + "_o", name=tagp + "_o"))
                n = len(specs)
                for k in range(n):
                    nc.vector.tensor_copy(tis[k][:], sps[k])
                for k in range(n):
                    nc.vector.tensor_copy(tfs[k][:], tis[k][:])
                for k in range(n):
                    nc.vector.tensor_tensor(ms[k][:], tfs[k][:], sps[k],
                                            Alu.is_gt)
                for k in range(n):
                    nc.vector.tensor_tensor(fls[k][:], tfs[k][:], ms[k][:],
                                            Alu.subtract)
                return [f[:] for f in fls]

            cx, cy, hw, hl = floors([(xq, "cx", None), (yq, "cy", None),
                                     (wq, "hw", 0.5), (lq, "hl", 0.5)])
            nc.gpsimd.tensor_scalar(hw, hw, 3.0, None, Alu.max)
            nc.gpsimd.tensor_scalar(hl, hl, 3.0, None, Alu.max)
            # (bounds on Pool below use only add/sub/min/max: Pool-legal)

            xmin = cpool.tile([128, SPC], fp32, tag="xmin")
            xmax = cpool.tile([128, SPC], fp32, tag="xmax")
            ymin = cpool.tile([128, SPC], fp32, tag="ymin")
            ymax = cpool.tile([128, SPC], fp32, tag="ymax")
            nc.gpsimd.tensor_tensor(xmin[:], cx, hw, Alu.subtract)
            nc.gpsimd.tensor_tensor(xmax[:], cx, hw, Alu.add)
            nc.gpsimd.tensor_tensor(ymin[:], cy, hl, Alu.subtract)
            nc.gpsimd.tensor_tensor(ymax[:], cy, hl, Alu.add)
            nc.gpsimd.tensor_scalar(xmin[:], xmin[:], 0.0, None, Alu.max)
            nc.gpsimd.tensor_scalar(xmax[:], xmax[:], 1.0, float(HO),
                                    Alu.add, Alu.min)
            nc.gpsimd.tensor_scalar(ymin[:], ymin[:], 0.0, None, Alu.max)
            nc.gpsimd.tensor_scalar(ymax[:], ymax[:], 1.0, float(WO),
                                    Alu.add, Alu.min)

            # validity * 2^(n-64), and z-weighted variant
            vw = cpool.tile([128, SPC], fp32, tag="vw")
            nc.vector.tensor_scalar(vw[:], wq, 0.0, None, Alu.is_gt)
            wS = cpool.tile([128, SPC], fp32, tag="wS")
            nc.vector.scalar_tensor_tensor(wS[:], lq, 0.0, vw[:],
                                           Alu.is_gt, Alu.logical_and)
            nc.vector.tensor_scalar(wS[:], wS[:], wS_base[:], None, Alu.mult)
            wA = cpool.tile([128, SPC], fp32, tag="wA")
            nc.vector.tensor_tensor(wA[:], wS[:], zq, Alu.mult)

            _ft = [0]

            def next_ftag():
                _ft[0] ^= 1
                return "F2" if _ft[0] else "F1"

            def emit_head(s, defer_dve=False):
                """DMA + resize step 1 + U/V prep for sample s.
                Returns ((out1a, out1b, U_s, U_a, Vb), dve_thunks)."""
                out1a = spool.tile([128, HO], bf16, tag="out1a")
                out1b = spool.tile([32, HO], bf16, tag="out1b")
                for msz, o1 in ((128, out1a), (32, out1b)):
                    moff = 0 if msz == 128 else 128
                    px = poolF.tile([128, 1024], fp32, tag=next_ftag())
                    for po, hs in ((slice(128, 512), slice(0, 384)),
                                   (slice(512, 768), slice(384, 640))):
                        nc.tensor.matmul(
                            px[0:msz, po],
                            F0all[:, s * W + moff:s * W + moff + msz],
                            A0[:, hs], start=True, stop=False)
                        nc.tensor.matmul(
                            px[0:msz, po],
                            F1all[:, s * W + moff:s * W + moff + msz],
                            A1[:, hs], start=False, stop=True)
                    nc.scalar.copy(o1[:, :], px[0:msz, 128:768])

                tlt = spool.tile([128, HO], f16, tag="tlt")
                tge = spool.tile([128, HO], f16, tag="tge")
                tlt2 = spool.tile([128, HO], f16, tag="tlt2")
                tge2 = spool.tile([128, HO], f16, tag="tge2")
                Uh = spool.tile([128, HO], f16, tag="Uh")
                Vb = spool.tile([128, HO], bf16, tag="Vb")
                U_s = spool.tile([128, HO], bf16, tag="Us")
                U_a = spool.tile([128, HO], bf16, tag="Ua")
                thunks = [
                    lambda: nc.vector.tensor_scalar(
                        tlt[:], iota_h[:], xmax[:, s:s + 1], None, Alu.is_lt),
                    lambda: nc.vector.tensor_scalar(
                        tge[:], iota_h[:], xmin[:, s:s + 1], None, Alu.is_ge),
                    lambda: nc.vector.tensor_scalar(
                        tlt2[:], iota_h[:], ymax[:, s:s + 1], None,
                        Alu.is_lt),
                    lambda: nc.vector.tensor_scalar(
                        tge2[:], iota_h[:], ymin[:, s:s + 1], None,
                        Alu.is_ge),
                    lambda: nc.vector.tensor_tensor(
                        Uh[:], tlt[:], tge[:], Alu.mult),
                    lambda: nc.vector.tensor_tensor(
                        Vb[:], tlt2[:], tge2[:], Alu.mult),
                    lambda: nc.vector.tensor_scalar(
                        U_s[:], Uh[:], wS[:, s:s + 1], None, Alu.mult),
                    lambda: nc.vector.tensor_scalar(
                        U_a[:], Uh[:], wA[:, s:s + 1], None, Alu.mult),
                ]
                if not defer_dve:
                    for t in thunks:
                        t()
                    thunks = []
                return (out1a, out1b, U_s, U_a, Vb), thunks

            def emit_tile(s, m, idx, hd, fillers):
                out1a, out1b, U_s, U_a, Vb = hd
                ms = slice(m * 128, (m + 1) * 128)

                TS_ = poolS.tile([128, HO], fp32, tag="SS")
                TA_ = poolA.tile([128, HO], fp32, tag="AA")
                TF_ = poolF.tile([128, 1024], fp32, tag=next_ftag())
                B512 = ((slice(0, 512), slice(0, 512)),
                        (slice(512, 640), slice(512, 640)))
                for po, hs in B512:
                    nc.tensor.matmul(TS_[:, po], eps_row[:], ones_row[:, hs],
                                     start=True, stop=False)
                    nc.tensor.matmul(TS_[:, po], U_s[:, ms], Vb[:, hs],
                                     start=False, stop=True)
                for po, hs in B512:
                    nc.tensor.matmul(TA_[:, po], U_a[:, ms], Vb[:, hs],
                                     start=True, stop=True)
                for po, hs in B512:
                    nc.tensor.matmul(TF_[:, po], out1a[:, ms], B0[:, hs],
                                     start=True, stop=False)
                    nc.tensor.matmul(TF_[:, po], out1b[:, ms], B1[:, hs],
                                     start=False, stop=False)

                # E = TS.bits & 0xFF800000  (isolate 2^top)
                E2 = dpool.tile([128, HO], i32, tag="E2")
                nc.vector.tensor_scalar(
                    E2[:], TS_[:].bitcast(i32),
                    MASK_EXP, None, Alu.bitwise_and)
                if fillers:
                    fillers.pop(0)()
                # Z.bits = (TA.bits + 0x3F800000) - E
                Z = dpool.tile([128, HO], fp32, tag="Z")
                nc.vector.scalar_tensor_tensor(
                    Z[:].bitcast(i32), TA_[:].bitcast(i32), XBIAS,
                    E2[:], Alu.add, Alu.subtract)
                if fillers:
                    fillers.pop(0)()
                # fp16 copy for the PE subtract (Pool is idle)
                Zh = dpool.tile([128, HO], f16, tag="Zh")
                nc.gpsimd.tensor_copy(Zh[:], Z[:])
                return TF_, Zh

            def emit_zsub_sq(TF_, Zh, idx):
                # PE: F -= Z  (fp16 identity matmul, closes the group)
                for po, hs in ((slice(0, 512), slice(0, 512)),
                               (slice(512, 640), slice(512, 640))):
                    nc.tensor.matmul(TF_[:, po], negI[:], Zh[:, hs],
                                     start=False, stop=True)
                # Act: accumulate (F - Z)^2 straight from PSUM
                dsq = dpool.tile([128, HO], fp32, tag="dsq")
                nc.scalar.activation(
                    dsq[:], TF_[:, 0:HO], AF.Square,
                    accum_out=accbuf[:, idx:idx + 1])

            for rep in range(krep):
                heads = {0: emit_head(0)[0]}
                fillers = []
                pending = None
                for s in range(SPC):
                    for m in range(5):
                        idx = ((rep * SPC + s) * 5) + m
                        if m == 0 and s + 1 < SPC:
                            hd2, th = emit_head(s + 1, defer_dve=True)
                            heads[s + 1] = hd2
                            fillers.extend(th)
                        cur = (emit_tile(s, m, idx, heads[s], fillers), idx)
                        if pending is not None:
                            (TRp, Zp), idxp = pending
                            emit_zsub_sq(TRp, Zp, idxp)
                        pending = cur
                    while fillers:
                        fillers.pop(0)()
                    del heads[s]
                if pending is not None:
                    (TRp, Zp), idxp = pending
                    emit_zsub_sq(TRp, Zp, idxp)
                    pending = None

            # ---- final reduction ----
            tot = cpool.tile([128, 1], fp32, tag="tot")
            nc.vector.tensor_reduce(
                tot[:], accbuf[:, 0:krep * SPC * 5],
                mybir.AxisListType.X, Alu.add)
            if krep > 1:
                nc.vector.tensor_scalar(tot[:], tot[:], 1.0 / krep, None,
                                        Alu.mult)
            pfin = poolS.tile([128, HO], fp32, tag="SS")
            nc.tensor.matmul(pfin[0:1, 0:1], tot[:], ones_col[:],
                             start=True, stop=True)
            res = cpool.tile([1, 1], fp32, tag="res")
            nc.scalar.copy(res[:], pfin[0:1, 0:1])
            nc.sync.dma_start(out_d.ap(), res[:])

    nc.compile()
    return nc


def _get_nc(krep=1):
    key = ("nc", krep)
    if key not in _CACHE:
        _CACHE[key] = _build(krep)
    return _CACHE[key]


def run_cores(feat, gt_bboxes, krep=1):
    """Run the SPMD kernel; returns list of per-core sum-of-squared-diffs."""
    import ml_dtypes
    from concourse.bass_utils import run_bass_kernel_spmd
    nc = _get_nc(krep)
    amat = _resize_matrix()
    amatb = amat.astype(ml_dtypes.bfloat16)
    feat = np.ascontiguousarray(np.asarray(feat, dtype=np.float32))
    gt = np.ascontiguousarray(np.asarray(gt_bboxes, dtype=np.float32))
    in_maps = []
    for i in range(NCORES):
        sl = slice(i * SPC, (i + 1) * SPC)
        in_maps.append({
            "feat": np.ascontiguousarray(feat[sl, 0]),
            "boxes": np.ascontiguousarray(gt[sl]),
            "amat": amat,
            "amatb": amatb,
        })
    res = run_bass_kernel_spmd(nc, in_maps, core_ids=list(range(NCORES)))
    return [float(res.results[i]["out"][0, 0]) for i in range(NCORES)]


def kernel(feat, gt_bboxes):
    parts = run_cores(feat, gt_bboxes, krep=1)
    total = float(np.sum(np.asarray(parts, dtype=np.float64)))
    return np.asarray(np.float32(total / NPIX))
